# revision 7
# baseline (speedup 1.0000x reference)
"""Trainium2 Bass kernel for nn_DAN_46943992545473 (segment_reduce).

reference:
  x = concat(emb_table[seq], pos_table[pos], axis=2)          # [B, S, 100]
  pooled = (x * (s < seq_length)).sum(s) / seq_length         # [B, 100]
  out = MLP(pooled)  (relu x3, linear)                        # [B, 2]

Strategy (8 cores, data-parallel on batch: 256 rows/core):
  The masked-mean of gathered embedding rows is a sparse-matrix product:
     pooled_emb = C @ emb_table,   C[b, v] = #{s < L_b : seq[b,s] = v}
     pooled_pos = P @ pos_table,   P[b, p] = #{s < L_b : pos[b,s] = p}
  The host builds C / P from the integer inputs; the device computes the
  products as chains of PE matmuls contracting vocab blocks of 128.
  C is uploaded as fp8e4 raw counts (exact for counts <= 16; host falls
  back to a bf16 1/L-folded variant otherwise) and used as the matmul
  *weights* so the fp8 fast-weight-load path applies; emb blocks stream.
  The 1/L scale rides the psum->SBUF ACT copy (per-partition, batch-major),
  then PE transposes restore the [dim, batch] layout the MLP wants.
  C streams from HBM in tapered chunks (small first chunk so PE starts
  early) on the sync queue, emb blocks on the vector queue, fused
  constants on the scalar queue. MLP runs transposed on PE, relu on ACT.
"""
import numpy as np
import ml_dtypes

import concourse.bacc as bacc
import concourse.bass as bass
import concourse.tile as tile
import concourse.mybir as mybir
from concourse.bass_utils import run_bass_kernel_spmd

# problem shapes (hardcoded per contract)
B, S = 2048, 512
VOCAB, MAXPOS = 50000, 512
DE = 50
DIN, H, OUT = 100, 512, 2
NCORES = 8
BL = B // NCORES            # 256 batches per core

CHUNKS = (14, 28, 50, 50, 50, 50, 50, 50, 36, 14)   # vocab blocks per chunk
NBV = sum(CHUNKS)           # 392 vocab blocks of 128
VPAD = NBV * 128            # 50176 (vocab padded)
NBS = MAXPOS // 128         # 4 pos blocks
NBH = BL // 128             # batch halves (2)

F32 = mybir.dt.float32
BF16 = mybir.dt.bfloat16
F8 = mybir.dt.float8e4
Act = mybir.ActivationFunctionType


def build_nc(mode="fp8"):
    fp8 = mode == "fp8"
    nc = bacc.Bacc("TRN2", target_bir_lowering=False, debug=False)
    d_emb = nc.dram_tensor("embp", [128, NBV * DE], BF16, kind="ExternalInput")
    d_ct = nc.dram_tensor("ctp", [128, NBV * BL], F8 if fp8 else BF16,
                          kind="ExternalInput")
    # fused small constants:
    #   pc   = pos blocks [128,4,50] ++ cpos blocks [128,4,256]  (bf16)
    #   w1f  = padded W1 [128,512] ++ Wf blocks [128,4,2]        (bf16)
    #   w23  = W2 blocks [128,4,512] ++ W3 blocks [128,4,512]    (bf16)
    #   bias = b1t|b2t|b3t [128,12] ++ rl [128,2] ++ bf [2,1]@col14 (f32)
    d_pc = nc.dram_tensor("pc", [128, NBS * (DE + BL)], BF16,
                          kind="ExternalInput")
    d_w1f = nc.dram_tensor("w1f", [128, H + NBS * OUT], BF16,
                           kind="ExternalInput")
    d_w23 = nc.dram_tensor("w23", [128, NBS * 2 * H], BF16,
                           kind="ExternalInput")
    d_bias = nc.dram_tensor("biasf", [128, 15], F32, kind="ExternalInput")
    d_id = nc.dram_tensor("ident", [128, 128], F32, kind="ExternalInput")
    d_out = nc.dram_tensor("outT", [OUT, BL], F32, kind="ExternalOutput")

    emb_ap = d_emb.ap().rearrange("p (k e) -> p k e", e=DE)
    ct_ap = d_ct.ap().rearrange("p (k b) -> p k b", b=BL)

    with tile.TileContext(nc) as tc:
        with (
            tc.tile_pool(name="const", bufs=1) as cp,
            tc.tile_pool(name="strm", bufs=3) as sp,
            tc.tile_pool(name="mlp", bufs=1) as mp,
            tc.tile_pool(name="psum", bufs=1, space="PSUM") as qp,
        ):
            # ---- constants on the scalar queue --------------------------
            pct = cp.tile([128, NBS, DE + BL], BF16, tag="pct")
            nc.scalar.dma_start(
                pct[:], d_pc.ap().rearrange("p (k f) -> p k f", f=DE + BL))
            w1f = mp.tile([128, H + NBS * OUT], BF16, tag="w1f")
            nc.scalar.dma_start(w1f[:], d_w1f.ap())
            w23 = mp.tile([128, NBS, 2 * H], BF16, tag="w23")
            nc.scalar.dma_start(
                w23[:], d_w23.ap().rearrange("p (k f) -> p k f", f=2 * H))
            biasf = cp.tile([128, 15], F32, tag="biasf")
            nc.scalar.dma_start(biasf[:], d_bias.ap())
            ident = cp.tile([128, 128], F32, tag="ident")
            nc.scalar.dma_start(ident[:], d_id.ap())
            w1t = w1f[:, 0:H]
            wft = w1f[:, H:].rearrange("p (k o) -> p k o", o=OUT)
            w2t = w23[:, :, 0:H]
            w3t = w23[:, :, H:2 * H]
            bts = [biasf[:, 0:4], biasf[:, 4:8], biasf[:, 8:12]]
            rlt = biasf[:, 12:14]
            bft = biasf[0:OUT, 14:15]

            pooled = mp.tile([128, BL], BF16, tag="pooled")
            nc.vector.memset(pooled[:], 0.0)

            # ---- emb pooled: stream C (sync q) + emb (vector q) ---------
            if fp8:
                # flipped: C is the (fp8, FWL) weight side, psum is [b, e]
                pe0 = qp.tile([128, DE], F32, tag="h2")
                pe1 = qp.tile([128, DE], F32, tag="h3")
                pes = [pe0, pe1]
            else:
                pemb = qp.tile([DE, BL], F32, tag="pemb")
            g0 = 0
            for c, chb in enumerate(CHUNKS):
                et = sp.tile([128, max(CHUNKS), DE], BF16, tag="et")
                nc.gpsimd.dma_start(et[:, 0:chb, :], emb_ap[:, g0:g0 + chb, :])
                ct = sp.tile([128, max(CHUNKS), BL], F8 if fp8 else BF16,
                             tag="ct")
                nc.sync.dma_start(ct[:, 0:chb, :], ct_ap[:, g0:g0 + chb, :])
                for k in range(chb):
                    gk = g0 + k
                    if fp8:
                        for h in range(NBH):
                            nc.tensor.matmul(
                                pes[h][:], ct[:, k, h * 128:(h + 1) * 128],
                                et[:, k, :], start=(gk == 0),
                                stop=(gk == NBV - 1))
                    else:
                        nc.tensor.matmul(pemb[:], et[:, k, :], ct[:, k, :],
                                         start=(gk == 0), stop=(gk == NBV - 1))
                g0 += chb

            if fp8:
                # 1/L scale on the psum->SBUF copy, then transpose to [e, b]
                for h in range(NBH):
                    he = mp.tile([128, DE], F32, tag=f"he{h}")
                    nc.scalar.activation(he[:], pes[h][:], Act.Identity,
                                         bias=0.0, scale=rlt[:, h:h + 1])
                    tr = qp.tile([DE, 128], F32, tag=f"h{h}")
                    nc.tensor.transpose(tr[:], he[:], ident[:])
                    nc.scalar.copy(pooled[0:DE, h * 128:(h + 1) * 128], tr[:])
            else:
                nc.scalar.copy(pooled[0:DE, :], pemb[:])

            # ---- pos pooled: 4-block matmul chain ([e, b] psum) ---------
            ppos = qp.tile([DE, BL], F32, tag="out")
            for k in range(NBS):
                nc.tensor.matmul(ppos[:], pct[:, k, 0:DE], pct[:, k, DE:],
                                 start=(k == 0), stop=(k == NBS - 1))
            nc.scalar.copy(pooled[64:64 + DE, :], ppos[:])

            # ---- MLP (transposed activations) ---------------------------
            hcur = pooled
            for li, (wt, bt) in enumerate(((w1t, bts[0]), (w2t, bts[1]),
                                           (w3t, bts[2]))):
                houts = []
                for m in range(H // 128):
                    ps = qp.tile([128, BL], F32, tag=f"h{m}")
                    if li == 0:
                        nc.tensor.matmul(ps[:], wt[:, m * 128:(m + 1) * 128],
                                         hcur[:], start=True, stop=True)
                    else:
                        for cc in range(H // 128):
                            nc.tensor.matmul(
                                ps[:], wt[:, cc, m * 128:(m + 1) * 128],
                                hcur[cc][:], start=(cc == 0),
                                stop=(cc == H // 128 - 1))
                    ht = mp.tile([128, BL], BF16, tag=f"a{li}m{m}")
                    nc.scalar.activation(ht[:], ps[:], Act.Relu,
                                         bias=bt[:, m:m + 1])
                    houts.append(ht)
                hcur = houts
            pso = qp.tile([OUT, BL], F32, tag="out")
            for cc in range(H // 128):
                nc.tensor.matmul(pso[:], wft[:, cc, :], hcur[cc][:],
                                 start=(cc == 0), stop=(cc == H // 128 - 1))
            outT = mp.tile([OUT, BL], F32, tag="outT")
            nc.scalar.activation(outT[:], pso[:], Act.Identity, bias=bft[:, :1])
            nc.sync.dma_start(d_out.ap(), outT[:])

    nc.compile()
    return nc


_NC_CACHE = {}


def _pad_w1(w1):
    wp = np.zeros((128, H), np.float32)
    wp[0:DE] = w1[0:DE]
    wp[64:64 + DE] = w1[DE:DIN]
    return wp


def _blockify(a, nblk, dtype):
    """[nblk*128, F] row-major -> [128, nblk, F] SBUF-partition-major."""
    f = a.shape[1]
    return np.ascontiguousarray(
        a.reshape(nblk, 128, f).transpose(1, 0, 2)).astype(dtype)


def _prep_shared(emb_table, pos_table, W1, b1, W2, b2, W3, b3, Wf, bf):
    bf16 = ml_dtypes.bfloat16
    emb_pad = np.zeros((VPAD, DE), np.float32)
    emb_pad[:VOCAB] = np.asarray(emb_table, np.float32)
    w1f = np.concatenate(
        [_pad_w1(np.asarray(W1, np.float32)),
         np.asarray(Wf, np.float32).reshape(NBS, 128, OUT)
         .transpose(1, 0, 2).reshape(128, NBS * OUT)], axis=1)
    w23 = np.concatenate(
        [_blockify(np.asarray(W2, np.float32), NBS, np.float32),
         _blockify(np.asarray(W3, np.float32), NBS, np.float32)],
        axis=2).reshape(128, NBS * 2 * H)
    return {
        "embp": _blockify(emb_pad, NBV, bf16).reshape(128, NBV * DE),
        "w1f": w1f.astype(bf16),
        "w23": w23.astype(bf16),
        "ident": np.eye(128, dtype=np.float32),
        "_posp": _blockify(np.asarray(pos_table, np.float32), NBS, np.float32),
        "_b123": np.stack([np.asarray(x, np.float32).reshape(NBS, 128).T
                           for x in (b1, b2, b3)], axis=1).reshape(128, 12),
        "_bf": np.asarray(bf, np.float32).reshape(OUT),
    }


def _count_matrix(idx, mask, width):
    """C.T: [width, BL] f32 with C[b, v] = #{s: mask[b,s] and idx[b,s]==v}."""
    bl = idx.shape[0]
    b_of = np.broadcast_to(np.arange(bl)[:, None], idx.shape)
    flat = idx[mask].astype(np.int64) * bl + b_of[mask]
    cnt = np.bincount(flat, minlength=width * bl).astype(np.float32)
    return cnt.reshape(width, bl)


def _run(inputs, trace=False):
    seq = np.asarray(inputs["seq"], np.int64)
    pos_i = np.asarray(inputs["pos"], np.int64)
    slen = np.asarray(inputs["seq_length"], np.int64)
    bf16 = ml_dtypes.bfloat16

    shared = _prep_shared(
        inputs["emb_table"], inputs["pos_table"], inputs["W1"], inputs["b1"],
        inputs["W2"], inputs["b2"], inputs["W3"], inputs["b3"],
        inputs["Wf"], inputs["bf"])
    hidden = {k: shared.pop(k) for k in list(shared) if k.startswith("_")}

    smask = np.arange(S)[None, :] < slen[:, None]       # [B, S]
    rl_all = (1.0 / slen).astype(np.float32)

    cts, cposs = [], []
    maxcnt = 0.0
    for i in range(NCORES):
        sl = slice(i * BL, (i + 1) * BL)
        cts.append(_count_matrix(seq[sl], smask[sl], VPAD))
        cposs.append(_count_matrix(pos_i[sl], smask[sl], MAXPOS))
        maxcnt = max(maxcnt, cts[-1].max())

    # counts are fp8e4-exact up to 16; fall back to bf16 otherwise
    mode = "fp8" if maxcnt <= 16 else "bf16"
    if mode not in _NC_CACHE:
        _NC_CACHE[mode] = build_nc(mode)
    nc = _NC_CACHE[mode]

    in_maps = []
    for i in range(NCORES):
        sl = slice(i * BL, (i + 1) * BL)
        rl = rl_all[sl]
        m = dict(shared)
        cpos = cposs[i] * rl[None, :]
        m["pc"] = np.concatenate(
            [hidden["_posp"], _blockify(cpos, NBS, np.float32)],
            axis=2).reshape(128, NBS * (DE + BL)).astype(bf16)
        biasf = np.zeros((128, 15), np.float32)
        biasf[:, 0:12] = hidden["_b123"]
        biasf[:, 12:14] = rl.reshape(NBH, 128).T
        biasf[0:OUT, 14] = hidden["_bf"]
        m["biasf"] = biasf
        if mode == "fp8":
            m["ctp"] = _blockify(cts[i], NBV, ml_dtypes.float8_e4m3).reshape(
                128, NBV * BL)
        else:
            m["ctp"] = _blockify(cts[i] * rl[None, :], NBV, bf16).reshape(
                128, NBV * BL)
        in_maps.append(m)

    res = run_bass_kernel_spmd(nc, in_maps, core_ids=list(range(NCORES)),
                               trace=trace)
    out = np.concatenate([res.results[i]["outT"].T for i in range(NCORES)],
                         axis=0)
    return np.ascontiguousarray(out, dtype=np.float32), res


def kernel(emb_table, pos_table, W1, b1, W2, b2, W3, b3, Wf, bf,
           seq, seq_length, pos):
    out, _ = _run(dict(emb_table=emb_table, pos_table=pos_table, W1=W1, b1=b1,
                       W2=W2, b2=b2, W3=W3, b3=b3, Wf=Wf, bf=bf, seq=seq,
                       seq_length=seq_length, pos=pos))
    return out


# revision 8
# speedup vs baseline: 1.1042x; 1.1042x over previous
"""Trainium2 Bass kernel for nn_DAN_46943992545473 (segment_reduce).

reference:
  x = concat(emb_table[seq], pos_table[pos], axis=2)          # [B, S, 100]
  pooled = (x * (s < seq_length)).sum(s) / seq_length         # [B, 100]
  out = MLP(pooled)  (relu x3, linear)                        # [B, 2]

Strategy (8 cores, data-parallel on batch: 256 rows/core):
  The masked-mean of gathered embedding rows is a sparse-matrix product:
     pooled_emb = C @ emb_table,   C[b, v] = #{s < L_b : seq[b,s] = v}
     pooled_pos = P @ pos_table,   P[b, p] = #{s < L_b : pos[b,s] = p}
  The host builds C / P from the integer inputs; the device computes the
  products as chains of PE matmuls contracting vocab blocks of 128.
  C is uploaded as fp8e4 raw counts (exact for counts <= 16; host falls
  back to a bf16 1/L-folded variant otherwise) and used as the matmul
  *weights* so the fp8 fast-weight-load path applies; emb blocks stream.
  The 1/L scale rides the psum->SBUF ACT copy (per-partition, batch-major),
  then PE transposes restore the [dim, batch] layout the MLP wants.
  C streams from HBM in tapered chunks (small first chunk so PE starts
  early) on the sync queue, emb blocks on the vector queue, fused
  constants on the scalar queue. MLP runs transposed on PE, relu on ACT.
"""
import numpy as np
import ml_dtypes

import concourse.bacc as bacc
import concourse.bass as bass
import concourse.tile as tile
import concourse.mybir as mybir
from concourse.bass_utils import run_bass_kernel_spmd

# problem shapes (hardcoded per contract)
B, S = 2048, 512
VOCAB, MAXPOS = 50000, 512
DE = 50
DIN, H, OUT = 100, 512, 2
NCORES = 8
BL = B // NCORES            # 256 batches per core

CHUNKS = (14, 28, 50, 50, 50, 50, 50, 50, 36, 14)   # vocab blocks per chunk
NBV = sum(CHUNKS)           # 392 vocab blocks of 128
VPAD = NBV * 128            # 50176 (vocab padded)
NBS = MAXPOS // 128         # 4 pos blocks
NBH = BL // 128             # batch halves (2)

F32 = mybir.dt.float32
BF16 = mybir.dt.bfloat16
F8 = mybir.dt.float8e4
Act = mybir.ActivationFunctionType


def build_nc(mode="fp8"):
    fp8 = mode == "fp8"
    nc = bacc.Bacc("TRN2", target_bir_lowering=False, debug=False)
    d_emb = nc.dram_tensor("embp", [128, NBV * DE], BF16, kind="ExternalInput")
    d_ct = nc.dram_tensor("ctp", [128, NBV * BL], F8 if fp8 else BF16,
                          kind="ExternalInput")
    # fused small constants:
    #   pc   = pos blocks [128,4,50] ++ cpos blocks [128,4,256]  (bf16)
    #   w1f  = padded W1 [128,512] ++ Wf blocks [128,4,2]        (bf16)
    #   w23  = W2 blocks [128,4,512] ++ W3 blocks [128,4,512]    (bf16)
    #   bias = b1t|b2t|b3t [128,12] ++ rl [128,2] ++ bf [2,1]@col14 (f32)
    d_pc = nc.dram_tensor("pc", [128, NBS * (DE + BL)], BF16,
                          kind="ExternalInput")
    d_w1f = nc.dram_tensor("w1f", [128, H + NBS * OUT], BF16,
                           kind="ExternalInput")
    d_w23 = nc.dram_tensor("w23", [128, NBS * 2 * H], BF16,
                           kind="ExternalInput")
    d_bias = nc.dram_tensor("biasf", [128, 15], F32, kind="ExternalInput")
    d_id = nc.dram_tensor("ident", [128, 128], F32, kind="ExternalInput")
    d_out = nc.dram_tensor("outT", [OUT, BL], F32, kind="ExternalOutput")

    emb_ap = d_emb.ap().rearrange("p (k e) -> p k e", e=DE)
    ct_ap = d_ct.ap().rearrange("p (k b) -> p k b", b=BL)

    with tile.TileContext(nc) as tc:
        with (
            tc.tile_pool(name="const", bufs=1) as cp,
            tc.tile_pool(name="strm", bufs=3) as sp,
            tc.tile_pool(name="mlp", bufs=1) as mp,
            tc.tile_pool(name="psum", bufs=1, space="PSUM") as qp,
        ):
            # ---- constants on the scalar queue --------------------------
            pct = cp.tile([128, NBS, DE + BL], BF16, tag="pct")
            nc.scalar.dma_start(
                pct[:], d_pc.ap().rearrange("p (k f) -> p k f", f=DE + BL))
            w1f = mp.tile([128, H + NBS * OUT], BF16, tag="w1f")
            nc.scalar.dma_start(w1f[:], d_w1f.ap())
            w23 = mp.tile([128, NBS, 2 * H], BF16, tag="w23")
            nc.scalar.dma_start(
                w23[:], d_w23.ap().rearrange("p (k f) -> p k f", f=2 * H))
            biasf = cp.tile([128, 15], F32, tag="biasf")
            nc.scalar.dma_start(biasf[:], d_bias.ap())
            ident = cp.tile([128, 128], F32, tag="ident")
            nc.scalar.dma_start(ident[:], d_id.ap())
            w1t = w1f[:, 0:H]
            wft = w1f[:, H:].rearrange("p (k o) -> p k o", o=OUT)
            w2t = w23[:, :, 0:H]
            w3t = w23[:, :, H:2 * H]
            bts = [biasf[:, 0:4], biasf[:, 4:8], biasf[:, 8:12]]
            rlt = biasf[:, 12:14]
            bft = biasf[0:OUT, 14:15]

            pooled = mp.tile([128, BL], BF16, tag="pooled")
            nc.vector.memset(pooled[:], 0.0)

            # ---- emb pooled: stream C (sync q) + emb (vector q) ---------
            if fp8:
                # flipped: C is the (fp8, FWL) weight side, psum is [b, e]
                pe0 = qp.tile([128, DE], F32, tag="h2")
                pe1 = qp.tile([128, DE], F32, tag="h3")
                pes = [pe0, pe1]
            else:
                pemb = qp.tile([DE, BL], F32, tag="pemb")
            g0 = 0
            for c, chb in enumerate(CHUNKS):
                et = sp.tile([128, max(CHUNKS), DE], BF16, tag="et")
                nc.sync.dma_start(et[:, 0:chb, :], emb_ap[:, g0:g0 + chb, :])
                ct = sp.tile([128, max(CHUNKS), BL], F8 if fp8 else BF16,
                             tag="ct")
                nc.sync.dma_start(ct[:, 0:chb, :], ct_ap[:, g0:g0 + chb, :])
                for k in range(chb):
                    gk = g0 + k
                    if fp8:
                        for h in range(NBH):
                            nc.tensor.matmul(
                                pes[h][:], ct[:, k, h * 128:(h + 1) * 128],
                                et[:, k, :], start=(gk == 0),
                                stop=(gk == NBV - 1))
                    else:
                        nc.tensor.matmul(pemb[:], et[:, k, :], ct[:, k, :],
                                         start=(gk == 0), stop=(gk == NBV - 1))
                g0 += chb

            if fp8:
                # 1/L scale on the psum->SBUF copy, then transpose to [e, b]
                for h in range(NBH):
                    he = mp.tile([128, DE], F32, tag=f"he{h}")
                    nc.scalar.activation(he[:], pes[h][:], Act.Identity,
                                         bias=0.0, scale=rlt[:, h:h + 1])
                    tr = qp.tile([DE, 128], F32, tag=f"h{h}")
                    nc.tensor.transpose(tr[:], he[:], ident[:])
                    nc.scalar.copy(pooled[0:DE, h * 128:(h + 1) * 128], tr[:])
            else:
                nc.scalar.copy(pooled[0:DE, :], pemb[:])

            # ---- pos pooled: 4-block matmul chain ([e, b] psum) ---------
            ppos = qp.tile([DE, BL], F32, tag="out")
            for k in range(NBS):
                nc.tensor.matmul(ppos[:], pct[:, k, 0:DE], pct[:, k, DE:],
                                 start=(k == 0), stop=(k == NBS - 1))
            nc.scalar.copy(pooled[64:64 + DE, :], ppos[:])

            # ---- MLP (transposed activations) ---------------------------
            hcur = pooled
            for li, (wt, bt) in enumerate(((w1t, bts[0]), (w2t, bts[1]),
                                           (w3t, bts[2]))):
                houts = []
                for m in range(H // 128):
                    ps = qp.tile([128, BL], F32, tag=f"h{m}")
                    if li == 0:
                        nc.tensor.matmul(ps[:], wt[:, m * 128:(m + 1) * 128],
                                         hcur[:], start=True, stop=True)
                    else:
                        for cc in range(H // 128):
                            nc.tensor.matmul(
                                ps[:], wt[:, cc, m * 128:(m + 1) * 128],
                                hcur[cc][:], start=(cc == 0),
                                stop=(cc == H // 128 - 1))
                    ht = mp.tile([128, BL], BF16, tag=f"a{li}m{m}")
                    nc.scalar.activation(ht[:], ps[:], Act.Relu,
                                         bias=bt[:, m:m + 1])
                    houts.append(ht)
                hcur = houts
            pso = qp.tile([OUT, BL], F32, tag="out")
            for cc in range(H // 128):
                nc.tensor.matmul(pso[:], wft[:, cc, :], hcur[cc][:],
                                 start=(cc == 0), stop=(cc == H // 128 - 1))
            outT = mp.tile([OUT, BL], F32, tag="outT")
            nc.scalar.activation(outT[:], pso[:], Act.Identity, bias=bft[:, :1])
            nc.sync.dma_start(d_out.ap(), outT[:])

    nc.compile()
    return nc


_NC_CACHE = {}


def _pad_w1(w1):
    wp = np.zeros((128, H), np.float32)
    wp[0:DE] = w1[0:DE]
    wp[64:64 + DE] = w1[DE:DIN]
    return wp


def _blockify(a, nblk, dtype):
    """[nblk*128, F] row-major -> [128, nblk, F] SBUF-partition-major."""
    f = a.shape[1]
    return np.ascontiguousarray(
        a.reshape(nblk, 128, f).transpose(1, 0, 2)).astype(dtype)


def _prep_shared(emb_table, pos_table, W1, b1, W2, b2, W3, b3, Wf, bf):
    bf16 = ml_dtypes.bfloat16
    emb_pad = np.zeros((VPAD, DE), np.float32)
    emb_pad[:VOCAB] = np.asarray(emb_table, np.float32)
    w1f = np.concatenate(
        [_pad_w1(np.asarray(W1, np.float32)),
         np.asarray(Wf, np.float32).reshape(NBS, 128, OUT)
         .transpose(1, 0, 2).reshape(128, NBS * OUT)], axis=1)
    w23 = np.concatenate(
        [_blockify(np.asarray(W2, np.float32), NBS, np.float32),
         _blockify(np.asarray(W3, np.float32), NBS, np.float32)],
        axis=2).reshape(128, NBS * 2 * H)
    return {
        "embp": _blockify(emb_pad, NBV, bf16).reshape(128, NBV * DE),
        "w1f": w1f.astype(bf16),
        "w23": w23.astype(bf16),
        "ident": np.eye(128, dtype=np.float32),
        "_posp": _blockify(np.asarray(pos_table, np.float32), NBS, np.float32),
        "_b123": np.stack([np.asarray(x, np.float32).reshape(NBS, 128).T
                           for x in (b1, b2, b3)], axis=1).reshape(128, 12),
        "_bf": np.asarray(bf, np.float32).reshape(OUT),
    }


def _count_matrix(idx, mask, width):
    """C.T: [width, BL] f32 with C[b, v] = #{s: mask[b,s] and idx[b,s]==v}."""
    bl = idx.shape[0]
    b_of = np.broadcast_to(np.arange(bl)[:, None], idx.shape)
    flat = idx[mask].astype(np.int64) * bl + b_of[mask]
    cnt = np.bincount(flat, minlength=width * bl).astype(np.float32)
    return cnt.reshape(width, bl)


def _run(inputs, trace=False):
    seq = np.asarray(inputs["seq"], np.int64)
    pos_i = np.asarray(inputs["pos"], np.int64)
    slen = np.asarray(inputs["seq_length"], np.int64)
    bf16 = ml_dtypes.bfloat16

    shared = _prep_shared(
        inputs["emb_table"], inputs["pos_table"], inputs["W1"], inputs["b1"],
        inputs["W2"], inputs["b2"], inputs["W3"], inputs["b3"],
        inputs["Wf"], inputs["bf"])
    hidden = {k: shared.pop(k) for k in list(shared) if k.startswith("_")}

    smask = np.arange(S)[None, :] < slen[:, None]       # [B, S]
    rl_all = (1.0 / slen).astype(np.float32)

    cts, cposs = [], []
    maxcnt = 0.0
    for i in range(NCORES):
        sl = slice(i * BL, (i + 1) * BL)
        cts.append(_count_matrix(seq[sl], smask[sl], VPAD))
        cposs.append(_count_matrix(pos_i[sl], smask[sl], MAXPOS))
        maxcnt = max(maxcnt, cts[-1].max())

    # counts are fp8e4-exact up to 16; fall back to bf16 otherwise
    mode = "fp8" if maxcnt <= 16 else "bf16"
    if mode not in _NC_CACHE:
        _NC_CACHE[mode] = build_nc(mode)
    nc = _NC_CACHE[mode]

    in_maps = []
    for i in range(NCORES):
        sl = slice(i * BL, (i + 1) * BL)
        rl = rl_all[sl]
        m = dict(shared)
        cpos = cposs[i] * rl[None, :]
        m["pc"] = np.concatenate(
            [hidden["_posp"], _blockify(cpos, NBS, np.float32)],
            axis=2).reshape(128, NBS * (DE + BL)).astype(bf16)
        biasf = np.zeros((128, 15), np.float32)
        biasf[:, 0:12] = hidden["_b123"]
        biasf[:, 12:14] = rl.reshape(NBH, 128).T
        biasf[0:OUT, 14] = hidden["_bf"]
        m["biasf"] = biasf
        if mode == "fp8":
            m["ctp"] = _blockify(cts[i], NBV, ml_dtypes.float8_e4m3).reshape(
                128, NBV * BL)
        else:
            m["ctp"] = _blockify(cts[i] * rl[None, :], NBV, bf16).reshape(
                128, NBV * BL)
        in_maps.append(m)

    res = run_bass_kernel_spmd(nc, in_maps, core_ids=list(range(NCORES)),
                               trace=trace)
    out = np.concatenate([res.results[i]["outT"].T for i in range(NCORES)],
                         axis=0)
    return np.ascontiguousarray(out, dtype=np.float32), res


def kernel(emb_table, pos_table, W1, b1, W2, b2, W3, b3, Wf, bf,
           seq, seq_length, pos):
    out, _ = _run(dict(emb_table=emb_table, pos_table=pos_table, W1=W1, b1=b1,
                       W2=W2, b2=b2, W3=W3, b3=b3, Wf=Wf, bf=bf, seq=seq,
                       seq_length=seq_length, pos=pos))
    return out
